# revision 1
# baseline (speedup 1.0000x reference)
"""VQ codebook cosine-similarity softmax kernel for Trainium2 (8 NeuronCores).

Computes softmax(cos_sim(batch, centroids)) for batch [131072, 1024] f32 and
centroids [256, 1024] f32, data-parallel over the batch dim across 8 cores.

Per-core pipeline (16384 rows):
  - SWDGE cast-DMA loads x tiles HBM f32 -> SBUF fp16 (halves SBUF traffic,
    enables full-rate fp16 matmuls; fp32 PSUM accumulation keeps rel err ~5e-5)
  - PE transposes each [128,128] fp16 block (x needs D on partitions for the
    matmul contraction); PSUM->SBUF copyback split between DVE and ACT
  - PE matmul: weights = xT block [128d,128n], moving = cnT [128d,256k],
    accumulating over 8 d-chunks into PSUM f32 [128n, 256k]
  - row norms on DVE: tensor_tensor_reduce(x*x) then rsqrt via the
    0x5f3759df bit trick + 3 Newton steps (keeps Ln/Sqrt off ACT so only
    the Exp table set ever loads)
  - softmax: logits = cos in [-1,1] so no max-subtraction needed;
    ACT Exp(scale=1/||x||) with accum_out giving the denominator,
    DVE reciprocal + ACT Copy(scale=1/denom) for the final normalize
"""

import os
import sys

if "/opt/trn_rl_repo" not in sys.path:
    sys.path.insert(0, "/opt/trn_rl_repo")

import numpy as np

RSQRT_MODE = os.environ.get("KM_RSQRT_MODE", "bit")  # bit | act
MUL_MODE = os.environ.get("KM_MUL_MODE", "act")  # act | dve
COPY_SPLIT = os.environ.get("KM_COPY_SPLIT", "0") == "1"  # DVE+ACT vs DVE only
SQ_MODE = os.environ.get("KM_SQ_MODE", "sts")  # sts | ttr | act
# NOTE: ttr (tensor_tensor_reduce) compiles and simulates fine but faults the
# device at runtime — do not use. sts (scalar_tensor_tensor + accum) works.
# SQ_SPLIT: columns of each row handled by DVE (sts); the rest go to ACT
# (Square). Both run at 1 elem/cycle/lane, so this splits the norm pass
# across the two engines. 0 = all ACT, 1024 = all DVE.
SQ_SPLIT = int(os.environ.get("KM_SQ_SPLIT", "640"))
EARLY_CLOSE = os.environ.get("KM_EARLY_CLOSE", "1") == "1"
SPS_BUFS = int(os.environ.get("KM_SPS_BUFS", "4"))
XT_BUFS = int(os.environ.get("KM_XT_BUFS", "4"))
E_BUFS = int(os.environ.get("KM_E_BUFS", "6"))
DEN_BUFS = int(os.environ.get("KM_DEN_BUFS", "6"))
NRM_BUFS = int(os.environ.get("KM_NRM_BUFS", "4"))


N, D, K = 131072, 1024, 256
NCORES = 8
NPC = N // NCORES  # rows per core
P = 128  # partitions / tile rows
XB = 4  # row-tiles per load/store DMA batch
G = 16  # row-tiles per norm group (batched rsqrt)
F1 = 832  # copyback columns done by DVE (rest by ACT)

RSQRT_MAGIC = 0x5F3759DF


def build_bass(npc=NPC):
    """Build the single-core SPMD program; every core runs this with its own
    x shard. Returns the compiled Bacc object."""
    from contextlib import ExitStack

    import concourse.bacc as bacc
    import concourse.mybir as mybir
    import concourse.tile as tile
    from concourse.masks import make_identity

    dt = mybir.dt
    AFT = mybir.ActivationFunctionType
    Alu = mybir.AluOpType

    nt = npc // P  # row tiles
    assert npc % (P * XB) == 0
    ngroups = (nt + G - 1) // G

    nc = bacc.Bacc(
        "TRN2", target_bir_lowering=False, debug=False, num_devices=NCORES
    )
    x_d = nc.dram_tensor("x", [npc, D], dt.float32, kind="ExternalInput")
    c_d = nc.dram_tensor("c", [K, D], dt.float32, kind="ExternalInput")
    o_d = nc.dram_tensor("o", [npc, K], dt.float32, kind="ExternalOutput")

    ND = D // P  # d-chunks (8)

    def emit_rsqrt(nc, dst, src, scratch_a, scratch_b, w):
        """dst[:, :w] = 1/sqrt(src[:, :w]).

        bit mode: 0x5f3759df bit trick + 3 Newton steps, all on DVE.
        act mode: exp(-0.5*ln(src)) seed on ACT + 1 Newton step on DVE.
        """
        if RSQRT_MODE == "bit":
            srci = src.bitcast(dt.int32)
            dsti = dst.bitcast(dt.int32)
            nc.vector.tensor_scalar(
                dsti, srci, 1, None, Alu.logical_shift_right
            )
            # magic - x == (x ^ 0xffffffff) + (magic + 1)  (avoids int negate)
            nc.vector.tensor_scalar(dsti, dsti, -1, None, Alu.bitwise_xor)
            nc.vector.tensor_scalar(dsti, dsti, RSQRT_MAGIC + 1, None, Alu.add)
            niter = 3
        else:
            nc.scalar.activation(scratch_a, src, AFT.Ln)
            nc.scalar.activation(dst, scratch_a, AFT.Exp, scale=-0.5)
            niter = 1
        for _ in range(niter):
            nc.vector.tensor_tensor(scratch_a, dst, dst, Alu.mult)
            nc.vector.tensor_tensor(scratch_b, scratch_a, src, Alu.mult)
            nc.vector.tensor_scalar(
                scratch_b, scratch_b, -0.5, 1.5, Alu.mult, Alu.add
            )
            nc.vector.tensor_tensor(dst, dst, scratch_b, Alu.mult)

    with tile.TileContext(nc) as tc, ExitStack() as ctx:
        const = ctx.enter_context(tc.tile_pool(name="const", bufs=1))
        ident = const.tile([P, P], dt.float16)
        make_identity(nc, ident[:])

        # cnT: [128 (d within chunk), ND * K] fp16; chunk b at cols [K*b, K*b+K)
        cnT = const.tile([P, ND * K], dt.float16)
        # per-tile squared row norms (partial sums: a=DVE part, b=ACT part)
        n2a = const.tile([P, max(nt, 1)], dt.float32)
        n2b = const.tile([P, max(nt, 1)], dt.float32)
        # per-tile softmax denominators and their reciprocals
        denscols = const.tile([P, max(nt, 1)], dt.float32)
        rdenscols = const.tile([P, max(nt, 1)], dt.float32)

        # ---- centroid prep (one-time, ~1MB); pools close before main loop ----
        with ExitStack() as _cstack:
            cctx = _cstack if EARLY_CLOSE else ctx
            cprep = cctx.enter_context(tc.tile_pool(name="cprep", bufs=2))
            cpsum = cctx.enter_context(
                tc.tile_pool(name="cpsum", bufs=2, space="PSUM")
            )
            for h in range(K // P):  # 2 halves of the K=256 centroids
                c32 = cprep.tile([P, D], dt.float32, tag="c32")
                nc.sync.dma_start(c32[:], c_d.ap()[P * h : P * (h + 1), :])
                csq = cprep.tile([P, D], dt.float32, tag="csq")
                cn2 = cprep.tile([P, 1], dt.float32, tag="cn2")
                if SQ_MODE == "ttr":
                    nc.vector.tensor_tensor_reduce(
                        csq[:], c32[:], c32[:], 1.0, 0.0, Alu.mult, Alu.add,
                        accum_out=cn2[:],
                    )
                elif SQ_MODE == "sts":
                    nc.vector.scalar_tensor_tensor(
                        csq[:], c32[:], 1.0, c32[:], Alu.mult, Alu.mult,
                        accum_out=cn2[:],
                    )
                else:
                    nc.scalar.activation(
                        csq[:], c32[:], AFT.Square, accum_out=cn2[:]
                    )
                crn = cprep.tile([P, 1], dt.float32, tag="crn")
                csa = cprep.tile([P, 1], dt.float32, tag="csa")
                csb = cprep.tile([P, 1], dt.float32, tag="csb")
                emit_rsqrt(nc, crn[:], cn2[:], csa[:], csb[:], 1)
                cn16 = cprep.tile([P, D], dt.float16, tag="cn16")
                nc.vector.tensor_scalar_mul(cn16[:], c32[:], crn[:])
                for b in range(ND):
                    pt = cpsum.tile([P, P], dt.float16, tag="ct_ps")
                    nc.tensor.transpose(
                        pt[:], cn16[:, P * b : P * (b + 1)], ident[:]
                    )
                    nc.vector.tensor_copy(
                        cnT[:, K * b + P * h : K * b + P * h + P], pt[:]
                    )

        # ---- main loop ----
        x16_pool = ctx.enter_context(tc.tile_pool(name="x16", bufs=2 * G // XB))
        xt_pool = ctx.enter_context(tc.tile_pool(name="xt", bufs=XT_BUFS))
        sq_pool = ctx.enter_context(tc.tile_pool(name="sq", bufs=2))
        e_pool = ctx.enter_context(tc.tile_pool(name="e", bufs=E_BUFS))
        pm_pool = ctx.enter_context(tc.tile_pool(name="pm", bufs=3))
        nrm_pool = ctx.enter_context(tc.tile_pool(name="nrm", bufs=NRM_BUFS))
        den_pool = ctx.enter_context(tc.tile_pool(name="den", bufs=DEN_BUFS))
        tps_pool = ctx.enter_context(
            tc.tile_pool(name="tps", bufs=2, space="PSUM")
        )
        sps_pool = ctx.enter_context(
            tc.tile_pool(name="sps", bufs=SPS_BUFS, space="PSUM")
        )

        for g in range(ngroups):
            t0 = g * G
            t1 = min(t0 + G, nt)
            gtiles = range(t0, t1)
            gw = t1 - t0
            # 1) cast-loads (XB row-tiles per DMA)
            xmacs = {}
            for tm in range(t0 // XB, (t1 + XB - 1) // XB):
                xm = x16_pool.tile([P, XB * D], dt.float16, tag="xm")
                src = x_d.ap()[P * XB * tm : P * XB * (tm + 1), :].rearrange(
                    "(s p) d -> p s d", s=XB
                )
                nc.gpsimd.dma_start(
                    xm[:].rearrange("p (s d) -> p s d", s=XB), src
                )
                xmacs[tm] = xm
            # 2) row norms^2, split column-wise across DVE (sts) and ACT (Square)
            sd = max(0, min(D, SQ_SPLIT))
            for t in gtiles:
                xm = xmacs[t // XB]
                xs = xm[:, D * (t % XB) : D * (t % XB + 1)]
                if sd > 0:
                    sqa = sq_pool.tile([P, D], dt.float16, tag="sqa")
                    nc.vector.scalar_tensor_tensor(
                        sqa[:, :sd], xs[:, :sd], 1.0, xs[:, :sd],
                        Alu.mult, Alu.mult, accum_out=n2a[:, t : t + 1],
                    )
                if sd < D:
                    sqb = sq_pool.tile([P, D], dt.float16, tag="sqb")
                    nc.scalar.activation(
                        sqb[:, sd:], xs[:, sd:], AFT.Square,
                        accum_out=n2b[:, t : t + 1],
                    )
            # 3) batched rsqrt for the group's norms
            rng = nrm_pool.tile([P, G], dt.float32, tag="rng")
            nsa = nrm_pool.tile([P, G], dt.float32, tag="nsa")
            nsb = nrm_pool.tile([P, G], dt.float32, tag="nsb")
            n2s = nrm_pool.tile([P, G], dt.float32, tag="n2s")
            if sd == 0:
                n2src = n2b[:, t0:t1]
            elif sd == D:
                n2src = n2a[:, t0:t1]
            else:
                nc.vector.tensor_tensor(
                    n2s[:, :gw], n2a[:, t0:t1], n2b[:, t0:t1], Alu.add
                )
                n2src = n2s[:, :gw]
            emit_rsqrt(nc, rng[:, :gw], n2src, nsa[:, :gw], nsb[:, :gw], gw)
            # 4) per XB-block: transpose -> matmul -> exp, then batched
            #    reciprocal of the denominators, normalize, store
            for tm in range(t0 // XB, (t1 + XB - 1) // XB):
                bt0 = max(t0, tm * XB)
                bt1 = min(t1, (tm + 1) * XB)
                for t in range(bt0, bt1):
                    xm = xmacs[t // XB]
                    xs = xm[:, D * (t % XB) : D * (t % XB + 1)]
                    tps = tps_pool.tile([P, D], dt.float16, tag="tps")
                    for b in range(ND):
                        nc.tensor.transpose(
                            tps[:, P * b : P * (b + 1)],
                            xs[:, P * b : P * (b + 1)],
                            ident[:],
                        )
                    xt = xt_pool.tile([P, D], dt.float16, tag="xt")
                    if COPY_SPLIT:
                        nc.vector.tensor_copy(xt[:, :F1], tps[:, :F1])
                        nc.scalar.copy(xt[:, F1:], tps[:, F1:])
                    else:
                        nc.vector.tensor_copy(xt[:], tps[:])
                    sps = sps_pool.tile([P, K], dt.float32, tag="sps")
                    for b in range(ND):
                        nc.tensor.matmul(
                            sps[:],
                            xt[:, P * b : P * (b + 1)],
                            cnT[:, K * b : K * (b + 1)],
                            start=(b == 0),
                            stop=(b == ND - 1),
                        )
                    e = e_pool.tile([P, K], dt.float32, tag="e")
                    den = den_pool.tile([P, 1], dt.float32, tag="den")
                    j = t - t0
                    nc.scalar.activation(
                        e[:], sps[:], AFT.Exp,
                        scale=rng[:, j : j + 1], accum_out=den[:],
                    )
                    rden = den_pool.tile([P, 1], dt.float32, tag="rden")
                    nc.vector.reciprocal(rden[:], den[:])
                    if t == bt0:
                        pm = pm_pool.tile([P, XB * K], dt.float32, tag="pmac")
                    if MUL_MODE == "act":
                        nc.scalar.activation(
                            pm[:, K * (t % XB) : K * (t % XB + 1)],
                            e[:], AFT.Copy, scale=rden[:],
                        )
                    else:
                        nc.vector.tensor_scalar_mul(
                            pm[:, K * (t % XB) : K * (t % XB + 1)],
                            e[:], rden[:],
                        )
                dst = o_d.ap()[
                    P * XB * tm : P * XB * (tm + 1), :
                ].rearrange("(s p) k -> p s k", s=XB)
                nc.sync.dma_start(
                    dst, pm[:].rearrange("p (s k) -> p s k", s=XB)
                )

    nc.compile()
    return nc


_cache = {}


def _get_nc(npc=NPC):
    if npc not in _cache:
        _cache[npc] = build_bass(npc)
    return _cache[npc]


def kernel(batch: np.ndarray, centroids: np.ndarray) -> np.ndarray:
    from concourse.bass_utils import run_bass_kernel_spmd

    assert batch.shape == (N, D) and centroids.shape == (K, D)
    batch = np.ascontiguousarray(batch, dtype=np.float32)
    centroids = np.ascontiguousarray(centroids, dtype=np.float32)

    nc = _get_nc()
    in_maps = [
        {"x": batch[i * NPC : (i + 1) * NPC], "c": centroids}
        for i in range(NCORES)
    ]
    res = run_bass_kernel_spmd(nc, in_maps, core_ids=list(range(NCORES)))
    return np.concatenate([res.results[i]["o"] for i in range(NCORES)], axis=0)



# revision 2
# speedup vs baseline: 1.4610x; 1.4610x over previous
"""VQ codebook cosine-similarity softmax kernel for Trainium2 (8 NeuronCores).

Computes softmax(cos_sim(batch, centroids)) for batch [131072, 1024] f32 and
centroids [256, 1024] f32, data-parallel over the batch dim across 8 cores.

Per-core pipeline (16384 rows, fp8 datapath; rel err ~5e-3 << 2e-2 tol):
  - SWDGE cast-DMA loads x HBM f32 -> SBUF fp8e4 in batches of XB=8 row
    tiles with a (p s) row interleave so every partition reads 32KB of
    contiguous HBM per load (fat descriptors -> near line-rate DMA)
  - PE transposes each [128,128] fp8 block (output element step 2, per the
    fp8-transpose hardware rule) into a [128, 2048] PSUM tile; DVE copies
    it back as dense u16 pairs (2x rate), garbage odd bytes carried along
  - PE DoubleRow fp8 matmuls: 4 instrs/tile, each contracting two d-chunks
    (lhsT [p,2,m] stride-2 views of xt, rhs [p,2,k] views of dense cnT),
    f32 PSUM accumulation -- half the moving cycles of fp16
  - row norms: DVE STT on cols [0,SQ_SPLIT) + ACT Square on the rest, both
    scaled so the accumulated n2 = (16*||x||)^2; group-batched bit-trick
    rsqrt (+3 Newton) then gives 1/(16*||x||) directly, which also undoes
    the 16x centroid scaling applied before fp8 quantization
  - norms run one 16-tile group AHEAD of the matmul/exp phase so ACT Exp
    never waits on a freshly computed norm batch
  - softmax: ACT Exp(scale=rng) -> e fp16 with f32 accum denominator;
    DVE batched reciprocal; DVE tensor_scalar normalize (all-fp16, 4x
    rate) into pm fp16; SWDGE cast-DMA stores pm fp16 -> HBM f32 with the
    same (p s) interleave (8KB contiguous per partition per store)
"""

import os
import sys

if "/opt/trn_rl_repo" not in sys.path:
    sys.path.insert(0, "/opt/trn_rl_repo")

import numpy as np

N, D, K = 131072, 1024, 256
NCORES = 8
NPC = N // NCORES  # rows per core
P = 128  # partitions / tile rows
ND = D // P  # d-chunks (8)

XB = int(os.environ.get("KM_XB", "8"))  # row-tiles per load/store DMA batch
G = int(os.environ.get("KM_G", "16"))  # row-tiles per norm group
PF = int(os.environ.get("KM_PF", "4"))  # load prefetch depth (batches)
SQ_SPLIT = int(os.environ.get("KM_SQ_SPLIT", "448"))  # norm cols on DVE
X8_BUFS = int(os.environ.get("KM_X8_BUFS", "6"))
XT_BUFS = int(os.environ.get("KM_XT_BUFS", "4"))
TPS_BUFS = int(os.environ.get("KM_TPS_BUFS", "3"))
SPS_BUFS = int(os.environ.get("KM_SPS_BUFS", "4"))
E_BUFS = int(os.environ.get("KM_E_BUFS", "10"))
PM_BUFS = int(os.environ.get("KM_PM_BUFS", "3"))

SC = 16.0  # centroid scale before fp8 quantization (keeps cn out of subnormals)
RSQRT_MAGIC = 0x5F3759DF


def build_bass(npc=NPC):
    from contextlib import ExitStack

    import concourse.bacc as bacc
    import concourse.mybir as mybir
    import concourse.tile as tile
    from concourse.masks import make_identity

    dt = mybir.dt
    AFT = mybir.ActivationFunctionType
    Alu = mybir.AluOpType
    DR = mybir.MatmulPerfMode.DoubleRow

    nt = npc // P  # row tiles (128)
    nb = nt // XB  # DMA batches (16)
    assert npc % (P * XB) == 0 and G % XB == 0 and nt % G == 0

    nc = bacc.Bacc(
        "TRN2", target_bir_lowering=False, debug=False, num_devices=NCORES
    )
    x_d = nc.dram_tensor("x", [npc, D], dt.float32, kind="ExternalInput")
    c_d = nc.dram_tensor("c", [K, D], dt.float32, kind="ExternalInput")
    o_d = nc.dram_tensor("o", [npc, K], dt.float32, kind="ExternalOutput")

    def t_out_view(t8, b):
        """Step-2 fp8 transpose output view for block b of a [P, 2048] tile."""
        return t8[:, 2 * P * b : 2 * P * (b + 1)].rearrange(
            "p (m x) -> p m x", x=2
        )[:, :, 0:1]

    def dr_lhsT(t8, q):
        """DoubleRow lhsT: blocks 2q,2q+1 (step-2) of a [P, 2048] tile."""
        return t8[:, 4 * P * q : 4 * P * (q + 1)].rearrange(
            "p (two m x) -> p two m x", two=2, x=2
        )[:, :, :, 0:1]

    def emit_rsqrt(nc, dst, src, sa, sb, w):
        """dst[:, :w] = 1/sqrt(src[:, :w]): bit trick + 3 Newton steps."""
        srci = src.bitcast(dt.int32)
        dsti = dst.bitcast(dt.int32)
        nc.vector.tensor_scalar(dsti, srci, 1, None, Alu.logical_shift_right)
        nc.vector.tensor_scalar(dsti, dsti, -1, None, Alu.bitwise_xor)
        nc.vector.tensor_scalar(dsti, dsti, RSQRT_MAGIC + 1, None, Alu.add)
        for _ in range(3):
            nc.vector.tensor_tensor(sa, dst, dst, Alu.mult)
            nc.vector.tensor_tensor(sb, sa, src, Alu.mult)
            nc.vector.tensor_scalar(sb, sb, -0.5, 1.5, Alu.mult, Alu.add)
            nc.vector.tensor_tensor(dst, dst, sb, Alu.mult)

    with tile.TileContext(nc) as tc, ExitStack() as ctx:
        const = ctx.enter_context(tc.tile_pool(name="const", bufs=1))
        ident = const.tile([P, P], dt.float8e4)
        make_identity(nc, ident[:])
        # cnT: dense fp8, d-chunk b at cols [K*b, K*b+K), pre-scaled by SC
        cnT = const.tile([P, ND * K], dt.float8e4)
        # per-tile scaled squared norms (a=DVE part, b=ACT part) and rsqrt
        n2a = const.tile([P, nt], dt.float32)
        n2b = const.tile([P, nt], dt.float32)
        rng = const.tile([P, nt], dt.float32)

        x8_pool = ctx.enter_context(tc.tile_pool(name="x8", bufs=X8_BUFS))
        xt_pool = ctx.enter_context(tc.tile_pool(name="xt", bufs=XT_BUFS))
        sqa_pool = ctx.enter_context(tc.tile_pool(name="sqa", bufs=2))
        sqb_pool = ctx.enter_context(tc.tile_pool(name="sqb", bufs=2))
        e_pool = ctx.enter_context(tc.tile_pool(name="e", bufs=E_BUFS))
        pm_pool = ctx.enter_context(tc.tile_pool(name="pm", bufs=PM_BUFS))
        den_pool = ctx.enter_context(tc.tile_pool(name="den", bufs=3))
        nrm_pool = ctx.enter_context(tc.tile_pool(name="nrm", bufs=2))
        tps_pool = ctx.enter_context(
            tc.tile_pool(name="tps", bufs=TPS_BUFS, space="PSUM")
        )
        sps_pool = ctx.enter_context(
            tc.tile_pool(name="sps", bufs=SPS_BUFS, space="PSUM")
        )
        cprep = ctx.enter_context(tc.tile_pool(name="cprep", bufs=2))
        cpsum = ctx.enter_context(tc.tile_pool(name="cpsum", bufs=1, space="PSUM"))

        # ---- x loads state ----
        xmacs = {}

        def issue_load(u):
            if u >= nb:
                return
            xm = x8_pool.tile([P, XB * D], dt.float8e4, tag="xm")
            src = x_d.ap()[P * XB * u : P * XB * (u + 1), :].rearrange(
                "(p s) d -> p s d", s=XB
            )
            nc.gpsimd.dma_start(xm[:].rearrange("p (s d) -> p s d", s=XB), src)
            xmacs[u] = xm

        def xtile(t):
            return xmacs[t // XB][:, D * (t % XB) : D * (t % XB + 1)]

        sd = max(0, min(D, SQ_SPLIT))

        def emit_norm(t):
            """Scaled norm^2 of tile t: n2 = (SC*||x_t||)^2, split DVE/ACT."""
            xs = xtile(t)
            if sd > 0:
                sqa = sqa_pool.tile([P, max(sd, 1)], dt.float16, tag="sqa")
                nc.vector.scalar_tensor_tensor(
                    sqa[:, :sd], xs[:, :sd], SC * SC, xs[:, :sd],
                    Alu.mult, Alu.mult, accum_out=n2a[:, t : t + 1],
                )
            if sd < D:
                sqb = sqb_pool.tile([P, D - sd], dt.float16, tag="sqb")
                nc.scalar.activation(
                    sqb[:], xs[:, sd:], AFT.Square, scale=SC,
                    accum_out=n2b[:, t : t + 1],
                )

        def emit_rsqrt_group(g):
            """rng[:, t] = 1/(SC*||x_t||) for the G tiles of group g."""
            t0 = g * G
            nsa = nrm_pool.tile([P, G], dt.float32, tag="nsa")
            nsb = nrm_pool.tile([P, G], dt.float32, tag="nsb")
            n2s = nrm_pool.tile([P, G], dt.float32, tag="n2s")
            if sd == 0:
                n2src = n2b[:, t0 : t0 + G]
            elif sd == D:
                n2src = n2a[:, t0 : t0 + G]
            else:
                nc.vector.tensor_tensor(
                    n2s[:], n2a[:, t0 : t0 + G], n2b[:, t0 : t0 + G], Alu.add
                )
                n2src = n2s[:]
            emit_rsqrt(nc, rng[:, t0 : t0 + G], n2src, nsa[:], nsb[:], G)

        # ---- prologue: prefetch loads, then centroid prep, then group-0
        # norms (loads were issued first so DMA streams from t=0) ----
        for u in range(min(PF, nb)):
            issue_load(u)

        for h in range(K // P):  # 2 halves of the K=256 centroids
            c32 = cprep.tile([P, D], dt.float32, tag="c32")
            nc.sync.dma_start(c32[:], c_d.ap()[P * h : P * (h + 1), :])
            csq = cprep.tile([P, D], dt.float32, tag="csq")
            cn2 = cprep.tile([P, 1], dt.float32, tag="cn2")
            nc.vector.scalar_tensor_tensor(
                csq[:], c32[:], 1.0, c32[:], Alu.mult, Alu.mult,
                accum_out=cn2[:],
            )
            crn = cprep.tile([P, 1], dt.float32, tag="crn")
            csa = cprep.tile([P, 1], dt.float32, tag="csa")
            csb = cprep.tile([P, 1], dt.float32, tag="csb")
            emit_rsqrt(nc, crn[:], cn2[:], csa[:], csb[:], 1)
            c8 = cprep.tile([P, D], dt.float8e4, tag="c8")
            nc.vector.tensor_scalar(
                c8[:], c32[:], crn[:], SC, Alu.mult, Alu.mult
            )
            ct = cpsum.tile([P, 2 * D], dt.float8e4, tag="ct_ps")
            for b in range(ND):
                nc.tensor.transpose(
                    t_out_view(ct, b), c8[:, P * b : P * (b + 1)], ident[:]
                )
                nc.vector.tensor_copy(
                    cnT[:, K * b + P * h : K * b + P * h + P], t_out_view(ct, b)
                )

        for t in range(min(G, nt)):
            emit_norm(t)
        emit_rsqrt_group(0)

        # ---- main loop over XB-batches; MM/Exp lag one tile behind the
        # transpose/copy emission so PE never stalls on the copyback ----
        pend = None  # tile whose matmul+exp emission is pending
        pend_xt = None

        def emit_mm_exp(t, xt, den_u):
            sps = sps_pool.tile([P, K], dt.float32, tag="sps")
            for q in range(ND // 2):
                rhs = cnT[:, 2 * K * q : 2 * K * (q + 1)].rearrange(
                    "p (two k) -> p two k", two=2
                )
                nc.tensor.matmul(
                    sps[:], dr_lhsT(xt, q), rhs,
                    start=(q == 0), stop=(q == ND // 2 - 1), perf_mode=DR,
                )
            e = e_pool.tile([P, K], dt.float16, tag="e")
            j = t % XB
            nc.scalar.activation(
                e[:], sps[:], AFT.Exp, scale=rng[:, t : t + 1],
                accum_out=den_u[:, j : j + 1],
            )
            return e

        for u in range(nb):
            issue_load(u + PF)
            den_u = den_pool.tile([P, XB], dt.float32, tag="den")
            es = {}
            for t in range(u * XB, (u + 1) * XB):
                xs = xtile(t)
                tps = tps_pool.tile([P, 2 * D], dt.float8e4, tag="tps")
                for b in range(ND):
                    nc.tensor.transpose(
                        t_out_view(tps, b), xs[:, P * b : P * (b + 1)], ident[:]
                    )
                xt = xt_pool.tile([P, 2 * D], dt.float8e4, tag="xt")
                nc.vector.tensor_copy(
                    xt[:].bitcast(dt.uint16), tps[:].bitcast(dt.uint16)
                )
                tn = t + G  # norms one group ahead
                if tn < nt:
                    emit_norm(tn)
                if pend is not None:
                    es[pend] = emit_mm_exp(pend, pend_xt, pend_den)
                pend, pend_xt, pend_den = t, xt, den_u
            if u % (G // XB) == G // XB - 1:
                g = u // (G // XB) + 1  # rsqrt for the group normed above
                if g * G < nt:
                    emit_rsqrt_group(g)
            # flush the pending tile so the batch's denominators complete
            es[pend] = emit_mm_exp(pend, pend_xt, pend_den)
            pend = None
            # batched reciprocal, normalize, store
            rden = den_pool.tile([P, XB], dt.float32, tag="rden")
            nc.vector.reciprocal(rden[:], den_u[:])
            pm = pm_pool.tile([P, XB * K], dt.float16, tag="pm")
            for t in range(u * XB, (u + 1) * XB):
                j = t % XB
                nc.vector.tensor_scalar_mul(
                    pm[:, K * j : K * (j + 1)], es[t][:], rden[:, j : j + 1]
                )
            dst = o_d.ap()[P * XB * u : P * XB * (u + 1), :].rearrange(
                "(p s) k -> p s k", s=XB
            )
            nc.gpsimd.dma_start(dst, pm[:].rearrange("p (s k) -> p s k", s=XB))

    nc.compile()
    return nc


_cache = {}


def _get_nc(npc=NPC):
    if npc not in _cache:
        _cache[npc] = build_bass(npc)
    return _cache[npc]


def kernel(batch: np.ndarray, centroids: np.ndarray) -> np.ndarray:
    from concourse.bass_utils import run_bass_kernel_spmd

    assert batch.shape == (N, D) and centroids.shape == (K, D)
    batch = np.ascontiguousarray(batch, dtype=np.float32)
    centroids = np.ascontiguousarray(centroids, dtype=np.float32)

    nc = _get_nc()
    in_maps = [
        {"x": batch[i * NPC : (i + 1) * NPC], "c": centroids}
        for i in range(NCORES)
    ]
    res = run_bass_kernel_spmd(nc, in_maps, core_ids=list(range(NCORES)))
    return np.concatenate([res.results[i]["o"] for i in range(NCORES)], axis=0)
